# revision 17
# baseline (speedup 1.0000x reference)
"""Trainium2 Bass kernel for nn_CrossAttention (B=4, C=256, H=W=64).

Per (batch, branch) the computation is an independent cross-attention:
    f = Wf @ other + bf          [32, 4096]
    g = Wg @ own   + bg          [32, 4096]
    h = Wh @ own   + bh          [256, 4096]
    S = f^T @ g                  [4096, 4096]
    att = softmax(S, axis=-1)    (normalize over columns m)
    sa[c, m] = sum_n h[c, n] * att[n, m]
    out = gamma * sa + own

There are B*2 = 8 independent problems -> one per NeuronCore (pure SPMD).

Key algebra: att[n, m] = E[n, m] / Z[n] with E = exp(S - K0), Z = rowsum(E),
so sa[c, m] = sum_n (h^T[n,c]/Z[n]) E[n,m].  E is computed ONCE (single exp
pass), kept SBUF-resident in bf16, with Z obtained for free via the
activation accum_out.  The K0 shift cancels exactly in E/Z and guards fp32
exp overflow.  The f/g/h compute path runs in fp16; E and h/Z use bf16.

Scheduling notes:
- The PE drops to half clock after ANY idle gap (3us ramp back), so the S
  PSUM is a 3-slot ring inside one [128, 3072] tile; each [128, 2048] chunk
  occupies two ring slots and its single exp reads both through a 2-level
  access pattern (one activation = one fixed overhead + one accum read),
  while the ring lets the next S pack start before the previous exp ends.
- 128-deep-contraction matmuls (sa chains, f/g/h convs) are split into two
  64-row PE bands at tile_position 0/64 that stream concurrently (disjoint
  input lanes), ~2x throughput.  The bottom band (rows 64-127) drains
  first, so it carries start=True (PSUM reset) and the top band carries
  stop=True: per-column reset-before-accumulate order holds.
- Input/weight/residual DMAs are spread over the SP/Pool/DVE queues; the
  residual is prefetched mid-pipeline; output stores go on two queues.
"""

import os
import sys

for _p in ("/opt/trn_rl_repo", "/opt/pypackages"):
    if _p not in sys.path:
        sys.path.insert(0, _p)

os.environ.setdefault("JAX_PLATFORMS", "")

import numpy as np

import concourse.bacc as bacc
import concourse.tile as tile
from concourse import mybir

F32 = mybir.dt.float32
F16 = mybir.dt.float16
BF16 = mybir.dt.bfloat16
AF = mybir.ActivationFunctionType

B, C, H, W = 4, 256, 64, 64
N = H * W            # 4096 pixels
C8 = C // 8          # 32
NT = N // 128        # 32 n-tiles
NGROUP = 4           # n-tiles per pipeline group
NG = NT // NGROUP    # 8 groups
MB = 512             # m-block (one PSUM bank of fp32)
NMB = N // MB        # 8 m-blocks
HALF = 2048          # E half-tile width
SUB = 1024           # S PSUM ring slot (2 banks)
K0 = 40.0            # constant subtracted inside exp (cancels in softmax)
ICH = 2048           # input DMA chunk columns
E_BUFS = 18          # rotating [128, 2048] bf16 E half-tiles
SA_SHIFT = 10        # sa trails the exp pipeline by this many slots
BANDS = False        # split 128-deep contractions into two 64-row PE bands


def build_bass():
    nc = bacc.Bacc()

    own_d = nc.dram_tensor("own16", [C, N], F16, kind="ExternalInput")
    oth_d = nc.dram_tensor("oth16", [C, N], F16, kind="ExternalInput")
    res_d = nc.dram_tensor("own32", [C, N], F32, kind="ExternalInput")
    wfg_d = nc.dram_tensor("wfg_t", [C, 256], F16, kind="ExternalInput")
    wh_d = nc.dram_tensor("wh_t", [C, C], F16, kind="ExternalInput")
    bfk_d = nc.dram_tensor("bfgk", [128, 3], F32, kind="ExternalInput")
    bh_d = nc.dram_tensor("bh_rep", [128, C], F32, kind="ExternalInput")
    gm_d = nc.dram_tensor("gamma_rep", [128, 1], F32, kind="ExternalInput")
    out_d = nc.dram_tensor("out", [C, N], F32, kind="ExternalOutput")

    NCH = N // ICH  # input chunks per partition-half

    with tile.TileContext(nc) as tc:
        with (
            tc.tile_pool(name="singles", bufs=1) as singles,
            tc.tile_pool(name="inp", bufs=1) as inp,
            tc.tile_pool(name="hxzp", bufs=NT) as hxzp,
            tc.tile_pool(name="epool", bufs=E_BUFS) as epool,
            tc.tile_pool(name="zpool", bufs=4) as zpool,
            tc.tile_pool(name="resp", bufs=2 * NMB) as resp,
            tc.tile_pool(name="outp", bufs=2) as outp,
            tc.tile_pool(name="ps_s", bufs=1, space="PSUM") as ps_s,
            tc.tile_pool(name="ps_sa", bufs=2, space="PSUM") as ps_sa,
        ):
            # ---- small constants ----
            # the f/g weights + biases + k0 ride the ACT queue (idle until the
            # first exp); wh/bh/gamma interleave on the SP queue after the
            # first own chunk; inputs: own on SP, oth on the Pool queue
            wfg_sb = [singles.tile([128, 256], F16, name=f"wfg{k}") for k in range(2)]
            wh_sb = [singles.tile([128, C], F16, name=f"wh{k}") for k in range(2)]
            bfk_sb = singles.tile([128, 3], F32)
            bh_sb = singles.tile([128, C], F32)
            gm_sb = singles.tile([128, 1], F32)
            nc.scalar.dma_start(out=bfk_sb, in_=bfk_d[:, :])
            for k in range(2):
                nc.scalar.dma_start(out=wfg_sb[k], in_=wfg_d[128 * k:128 * (k + 1), :])
            wf_sb = [w[:, 0:128] for w in wfg_sb]
            wg_sb = [w[:, 128:256] for w in wfg_sb]
            bf_sb = bfk_sb[:, 0:1]
            bg_sb = bfk_sb[:, 1:2]
            k0_sb = bfk_sb[:, 2:3]

            own_sb = [[inp.tile([128, ICH], F16, name=f"own{k}_{c}")
                       for c in range(NCH)] for k in range(2)]
            oth_sb = [[inp.tile([128, ICH], F16, name=f"oth{k}_{c}")
                       for c in range(NCH)] for k in range(2)]
            for k in range(2):
                nc.sync.dma_start(
                    out=own_sb[k][0],
                    in_=own_d[128 * k:128 * (k + 1), 0:ICH])
                nc.gpsimd.dma_start(
                    out=oth_sb[k][0],
                    in_=oth_d[128 * k:128 * (k + 1), 0:ICH])
            for k in range(2):
                nc.sync.dma_start(out=wh_sb[k], in_=wh_d[128 * k:128 * (k + 1), :])
            nc.sync.dma_start(out=bh_sb, in_=bh_d[:, :])
            nc.sync.dma_start(out=gm_sb, in_=gm_d[:, :])
            for c in range(1, NCH):
                for k in range(2):
                    nc.sync.dma_start(
                        out=own_sb[k][c],
                        in_=own_d[128 * k:128 * (k + 1), ICH * c:ICH * (c + 1)])
                    nc.gpsimd.dma_start(
                        out=oth_sb[k][c],
                        in_=oth_d[128 * k:128 * (k + 1), ICH * c:ICH * (c + 1)])

            # f/g as per-m-block tiles (dependency granularity lets group 0's
            # stats overlap the conv tail); 4 partition-group replicas each.
            f_q = [singles.tile([128, MB], F16, name=f"f{nb}") for nb in range(NMB)]
            g_q = [singles.tile([128, MB], F16, name=f"g{nb}") for nb in range(NMB)]
            sa_sb = [singles.tile([128, N], F32, name=f"sa{k}") for k in range(2)]
            hxz = [hxzp.tile([128, C], BF16, name=f"hxz{i}", tag="hxz")
                   for i in range(NT)]
            xr_sb = {}

            def conv_fg(dst, w_sb, src, b_sb, nb, ps=None):
                # weights pre-tiled 4x along output columns: the conv output
                # lands replicated in all four partition quadrants
                c, o = (MB * nb) // ICH, (MB * nb) % ICH
                if ps is None:
                    ps = ps_sa.tile([128, MB], F32, tag="sa")
                first = True
                bands = (1, 0) if BANDS else (None,)
                for k in range(2):
                    for b in bands:
                        sl = slice(None) if b is None else slice(64 * b, 64 * (b + 1))
                        nc.tensor.matmul(
                            out=ps,
                            lhsT=w_sb[k][sl, :],
                            rhs=src[k][c][sl, o:o + MB],
                            start=first,
                            stop=(k == 1 and b in (0, None)),
                            tile_position=(0 if b is None else 64 * b, 0),
                        )
                        first = False
                nc.vector.tensor_scalar(
                    out=dst[nb],
                    in0=ps,
                    scalar1=b_sb[0:128, 0:1],
                    scalar2=None,
                    op0=mybir.AluOpType.add,
                )

            def conv_h(i):
                c, o = (128 * i) // ICH, (128 * i) % ICH
                ph = ps_sa.tile([128, MB], F32, tag="sa")
                first = True
                bands = (1, 0) if BANDS else (None,)
                for k in range(2):
                    for b in bands:
                        sl = slice(None) if b is None else slice(64 * b, 64 * (b + 1))
                        nc.tensor.matmul(
                            out=ph[:, 0:C],
                            lhsT=own_sb[k][c][sl, o:o + 128],
                            rhs=wh_sb[k][sl, :],
                            start=first,
                            stop=(k == 1 and b in (0, None)),
                            tile_position=(0 if b is None else 64 * b, 0),
                        )
                        first = False
                # bias folded into the PSUM->SBUF cast (bh replicated on host)
                nc.vector.tensor_tensor(
                    out=hxz[i],
                    in0=ph[:, 0:C],
                    in1=bh_sb,
                    op=mybir.AluOpType.add,
                )

            # E half-tiles for the in-flight groups: e_half[g % 3][a][h]
            e_half = [[[None] * 2 for _ in range(NGROUP)] for _ in range(3)]

            # S PSUM: one [128, 3*SUB] tile used as a 3-slot ring of
            # [128, SUB] sub-chunks (6 banks).  Each chunk occupies two ring
            # slots; the single exp reads both through a 2-level access
            # pattern, so the full [128, 2048] chunk costs ONE activation
            # (one fixed overhead + one accum read) while the ring still
            # lets the next S pack start before the previous exp finishes.
            s_ring = ps_s.tile([128, 3 * SUB], F32, name="s_ring")
            s_r3 = s_ring.rearrange("p (c m) -> p c m", c=3)
            ring_pair = {0: s_r3[:, 0:2, :], 2: s_r3[:, 2::-2, :],
                         1: s_r3[:, 1:3, :]}
            chunk_ctr = [0]

            def stats_chunk(g, a, h, zp):
                """S chunk (n-tile 4g+a, m half h) -> exp -> E + Z part."""
                i = NGROUP * g + a
                nb, o = i // NGROUP, 128 * (i % NGROUP)
                et = epool.tile([128, HALF], BF16, name=f"e{g}_{a}_{h}", tag="e")
                e_half[g % 3][a][h] = et
                k = chunk_ctr[0]
                chunk_ctr[0] += 1
                r0 = (2 * k) % 3
                for s in range(2):
                    r = (2 * k + s) % 3
                    for jj in range(2):
                        j = 2 * s + jj
                        nc.tensor.matmul(
                            out=s_r3[:, r, MB * jj:MB * (jj + 1)],
                            lhsT=f_q[nb][32 * j:32 * (j + 1), o:o + 128],
                            rhs=g_q[4 * h + j][32 * j:32 * (j + 1), :],
                            start=True,
                            stop=True,
                            tile_position=(32 * j, 0),
                        )
                nc.scalar.activation(
                    out=et,
                    in_=ring_pair[r0],
                    func=AF.Exp,
                    bias=k0_sb[:, 0:1],
                    accum_out=zp[:, 2 * a + h:2 * a + h + 1],
                )

            def zprep(g, zp):
                """Z = sum of the two half-sums; hxz *= 1/Z (in place)."""
                zt = zpool.tile([128, NGROUP], F32, tag="zt")
                rz = zpool.tile([128, NGROUP], F32, tag="rz")
                nc.vector.tensor_add(out=zt, in0=zp[:, 0:8:2], in1=zp[:, 1:8:2])
                nc.vector.reciprocal(out=rz, in_=zt)
                # gamma folded here: hxz = h * (gamma / Z), so sa_sb directly
                # accumulates gamma*sa and the epilogue is a single add
                nc.vector.tensor_scalar(
                    out=rz, in0=rz, scalar1=gm_sb[:, 0:1], scalar2=None,
                    op0=mybir.AluOpType.mult,
                )
                for a in range(NGROUP):
                    nc.vector.tensor_scalar(
                        out=hxz[NGROUP * g + a],
                        in0=hxz[NGROUP * g + a],
                        scalar1=rz[:, a:a + 1],
                        scalar2=None,
                        op0=mybir.AluOpType.mult,
                    )

            def sa_mb(g, mb):
                """Accumulate group g's contribution to sa[:, mb block]."""
                h = mb // (NMB // 2)
                m0 = MB * mb - HALF * h
                pas = []
                for ch in range(2):
                    pa = ps_sa.tile([128, MB], F32, tag="sa")
                    first = True
                    bands = (1, 0) if BANDS else (None,)
                    for a in range(NGROUP):
                        et = e_half[g % 3][a][h]
                        hx = hxz[NGROUP * g + a]
                        for b in bands:
                            sl = slice(None) if b is None else slice(64 * b, 64 * (b + 1))
                            nc.tensor.matmul(
                                out=pa,
                                lhsT=hx[sl, 128 * ch:128 * (ch + 1)],
                                rhs=et[sl, m0:m0 + MB],
                                start=first,
                                stop=(a == NGROUP - 1 and b in (0, None)),
                                tile_position=(0 if b is None else 64 * b, 0),
                            )
                            first = False
                    dst = sa_sb[ch][:, MB * mb:MB * (mb + 1)]
                    if g == 0:
                        # seed with the residual: sa_sb = gamma*sa_0 + x
                        nc.vector.tensor_add(out=dst, in0=pa,
                                             in1=xr_sb[(mb, ch)])
                    elif g < NG - 1:
                        nc.vector.tensor_add(out=dst, in0=dst, in1=pa)
                    else:
                        pas.append(pa)
                return pas

            def prefetch_xr(mb):
                for ch in range(2):
                    xr = resp.tile([128, MB], F32, tag="xr")
                    xr_sb[(mb, ch)] = xr
                    nc.gpsimd.dma_start(
                        out=xr,
                        in_=res_d[128 * ch:128 * (ch + 1), MB * mb:MB * (mb + 1)])

            def epilogue_mb(mb, pa_last):
                # final group's PSUM merged straight into the output tile
                for ch in range(2):
                    ot = outp.tile([128, MB], F32, tag="ot")
                    nc.vector.tensor_add(
                        out=ot,
                        in0=sa_sb[ch][:, MB * mb:MB * (mb + 1)],
                        in1=pa_last[ch],
                    )
                    eng = nc.sync if ch == 0 else nc.gpsimd
                    eng.dma_start(
                        out=out_d[128 * ch:128 * (ch + 1), MB * mb:MB * (mb + 1)],
                        in_=ot,
                    )

            # ---- slot schedule ----
            # Conv work not needed before the first stats chunk becomes the
            # trailing filler for the first SA_SHIFT slots; afterwards the
            # trailing sa m-blocks (lagging SA_SHIFT slots) fill that role.
            filler = [("g", nb) for nb in range(4, NMB)] \
                   + [("f", nb) for nb in range(1, NMB)] \
                   + [("x", mb) for mb in range(4)] \
                   + [("h", i) for i in range(NT)] \
                   + [("x", mb) for mb in range(4, NMB)]
            # upfront: everything the first stats chunk (n-tile 0, half 0)
            # needs: f block 0 and g blocks 0..3.  These use the (still idle)
            # S-ring PSUM regions so they pipeline without pool ping-pong.
            conv_fg(f_q, wf_sb, oth_sb, bf_sb, 0, ps=s_r3[:, 0, 0:MB])
            for nb in range(4):
                r, half = (1 + nb) // 2, (1 + nb) % 2
                conv_fg(g_q, wg_sb, own_sb, bg_sb, nb,
                        ps=s_r3[:, r, MB * half:MB * (half + 1)])

            fill_per_slot = (len(filler) + SA_SHIFT - 1) // SA_SHIFT
            zps = {}

            def emit_slot_filler(pos):
                sidx = pos - SA_SHIFT
                if sidx >= 0:
                    sg, smb = sidx // NMB, sidx % NMB
                    pas = sa_mb(sg, smb)
                    if sg == NG - 1:
                        epilogue_mb(smb, pas)
                # zprep AFTER the slot's sa adds: its DVE burst must not
                # delay the adds that the next sa chains ping-pong on
                if sidx >= -1 and (sidx + 1) % NMB == 0:
                    zg = (sidx + 1) // NMB
                    if zg in zps:
                        zprep(zg, zps.pop(zg))
                else:
                    for _ in range(fill_per_slot):
                        if filler:
                            kind, arg = filler.pop(0)
                            if kind == "g":
                                conv_fg(g_q, wg_sb, own_sb, bg_sb, arg)
                            elif kind == "f":
                                conv_fg(f_q, wf_sb, oth_sb, bf_sb, arg)
                            elif kind == "h":
                                conv_h(arg)
                            else:
                                prefetch_xr(arg)

            chunks = [(a, h) for h in range(2) for a in range(NGROUP)]
            for g in range(NG):
                zps[g] = zpool.tile([128, 2 * NGROUP], F32, tag="zp", name=f"zp{g}")
                for k, (a, h) in enumerate(chunks):
                    stats_chunk(g, a, h, zps[g])
                    emit_slot_filler(g * 8 + k)
            for pos in range(NG * 8, NG * 8 + SA_SHIFT):
                emit_slot_filler(pos)

    # run_bass_via_pjrt binds the exec primitive directly and never
    # finalizes; Bacc's register allocation + matmul-wait splitting live in
    # finalize()/compile(), so run it here.
    if not nc.is_finalized():
        nc.finalize()
    return nc


_NC_CACHE = None


def _get_nc():
    global _NC_CACHE
    if _NC_CACHE is None:
        _NC_CACHE = build_bass()
    return _NC_CACHE


def make_in_maps(**inputs):
    """Build the 8 per-core input maps (core 2b = x-branch, 2b+1 = y-branch)."""
    f = lambda a: np.ascontiguousarray(np.asarray(a), dtype=np.float32)
    h16 = lambda a: np.ascontiguousarray(np.asarray(a), dtype=np.float16)
    x = f(inputs["x"]).reshape(B, C, N)
    y = f(inputs["y"]).reshape(B, C, N)
    x16, y16 = x.astype(np.float16), y.astype(np.float16)
    Wfx, bfx = h16(inputs["Wfx"]), f(inputs["bfx"])
    Wgx, bgx = h16(inputs["Wgx"]), f(inputs["bgx"])
    Whx, bhx = h16(inputs["Whx"]), f(inputs["bhx"])
    Wfy, bfy = h16(inputs["Wfy"]), f(inputs["bfy"])
    Wgy, bgy = h16(inputs["Wgy"]), f(inputs["bgy"])
    Why, bhy = h16(inputs["Why"]), f(inputs["bhy"])
    gamma = f(inputs["gamma"])

    rep4 = lambda b: np.ascontiguousarray(np.tile(b, 4).reshape(128, 1))
    gam = np.ascontiguousarray(np.broadcast_to(gamma.reshape(1, 1), (128, 1)))

    # conv weights transposed AND 4x-tiled along output columns so the conv
    # lands replicated in the four partition quadrants
    c16 = lambda a: np.ascontiguousarray(a, dtype=np.float16)
    t4 = lambda Wt: c16(np.tile(Wt.T, (1, 4)))  # [C8, C] -> [C, 128]
    repb = lambda b: np.ascontiguousarray(
        np.broadcast_to(b.reshape(1, C), (128, C)), dtype=np.float32)
    k0_col = np.full((128, 1), -K0, np.float32)
    pack3 = lambda bf, bg: np.ascontiguousarray(
        np.concatenate([rep4(bf), rep4(bg), k0_col], axis=1), dtype=np.float32)
    packw = lambda Wf, Wg: np.ascontiguousarray(
        np.concatenate([t4(Wf), t4(Wg)], axis=1), dtype=np.float16)
    branch = {
        "x": dict(
            wfg_t=packw(Wfy, Wgx), wh_t=c16(Whx.T),
            bfgk=pack3(bfy, bgx), bh_rep=repb(bhx),
        ),
        "y": dict(
            wfg_t=packw(Wfx, Wgy), wh_t=c16(Why.T),
            bfgk=pack3(bfx, bgy), bh_rep=repb(bhy),
        ),
    }

    in_maps = []
    for b in range(B):
        in_maps.append(dict(own16=x16[b], oth16=y16[b], own32=x[b],
                            gamma_rep=gam, **branch["x"]))
        in_maps.append(dict(own16=y16[b], oth16=x16[b], own32=y[b],
                            gamma_rep=gam, **branch["y"]))
    return in_maps


def kernel(**inputs):
    from concourse.bass_utils import run_bass_kernel_spmd

    nc = _get_nc()
    in_maps = make_in_maps(**inputs)
    res = run_bass_kernel_spmd(nc, in_maps, list(range(8))).results
    out_x = np.stack([res[2 * b]["out"] for b in range(B)]).reshape(B, C, H, W)
    out_y = np.stack([res[2 * b + 1]["out"] for b in range(B)]).reshape(B, C, H, W)
    return (out_x, out_y)


# revision 19
# speedup vs baseline: 1.0187x; 1.0187x over previous
"""Trainium2 Bass kernel for nn_CrossAttention (B=4, C=256, H=W=64).

Per (batch, branch) the computation is an independent cross-attention:
    f = Wf @ other + bf          [32, 4096]
    g = Wg @ own   + bg          [32, 4096]
    h = Wh @ own   + bh          [256, 4096]
    S = f^T @ g                  [4096, 4096]
    att = softmax(S, axis=-1)    (normalize over columns m)
    sa[c, m] = sum_n h[c, n] * att[n, m]
    out = gamma * sa + own

There are B*2 = 8 independent problems -> one per NeuronCore (pure SPMD).

Key algebra: att[n, m] = E[n, m] / Z[n] with E = exp(S - K0), Z = rowsum(E),
so sa[c, m] = sum_n (h^T[n,c]/Z[n]) E[n,m].  E is computed ONCE (single exp
pass), kept SBUF-resident in bf16, with Z obtained for free via the
activation accum_out.  The K0 shift cancels exactly in E/Z and guards fp32
exp overflow.  The f/g/h compute path runs in fp16; E and h/Z use bf16.

Scheduling notes:
- The PE drops to half clock after ANY idle gap (3us ramp back), so the S
  PSUM is a 3-slot ring inside one [128, 3072] tile; each [128, 2048] chunk
  occupies two ring slots and its single exp reads both through a 2-level
  access pattern (one activation = one fixed overhead + one accum read),
  while the ring lets the next S pack start before the previous exp ends.
- 128-deep-contraction matmuls (sa chains, f/g/h convs) are split into two
  64-row PE bands at tile_position 0/64 that stream concurrently (disjoint
  input lanes), ~2x throughput.  The bottom band (rows 64-127) drains
  first, so it carries start=True (PSUM reset) and the top band carries
  stop=True: per-column reset-before-accumulate order holds.
- Input/weight/residual DMAs are spread over the SP/Pool/DVE queues; the
  residual is prefetched mid-pipeline; output stores go on two queues.
"""

import os
import sys

for _p in ("/opt/trn_rl_repo", "/opt/pypackages"):
    if _p not in sys.path:
        sys.path.insert(0, _p)

os.environ.setdefault("JAX_PLATFORMS", "")

import numpy as np

import concourse.bacc as bacc
import concourse.tile as tile
from concourse import mybir

F32 = mybir.dt.float32
F16 = mybir.dt.float16
BF16 = mybir.dt.bfloat16
AF = mybir.ActivationFunctionType

B, C, H, W = 4, 256, 64, 64
N = H * W            # 4096 pixels
C8 = C // 8          # 32
NT = N // 128        # 32 n-tiles
NGROUP = 4           # n-tiles per pipeline group
NG = NT // NGROUP    # 8 groups
MB = 512             # m-block (one PSUM bank of fp32)
NMB = N // MB        # 8 m-blocks
HALF = 2048          # E half-tile width
SUB = 1024           # S PSUM ring slot (2 banks)
K0 = 40.0            # constant subtracted inside exp (cancels in softmax)
ICH = 2048           # input DMA chunk columns
E_BUFS = 20          # rotating [128, 2048] bf16 E half-tiles
SA_SHIFT = 10        # sa trails the exp pipeline by this many slots
BANDS = False        # split 128-deep contractions into two 64-row PE bands


def build_bass():
    nc = bacc.Bacc()

    own_d = nc.dram_tensor("own16", [C, N], F16, kind="ExternalInput")
    oth_d = nc.dram_tensor("oth16", [C, N], F16, kind="ExternalInput")
    res_d = nc.dram_tensor("own32", [C, N], F32, kind="ExternalInput")
    wfg_d = nc.dram_tensor("wfg_t", [C, 256], F16, kind="ExternalInput")
    wh_d = nc.dram_tensor("wh_t", [C, C], F16, kind="ExternalInput")
    bfk_d = nc.dram_tensor("bfgk", [128, 3], F32, kind="ExternalInput")
    bh_d = nc.dram_tensor("bh_rep", [128, C], F32, kind="ExternalInput")
    gm_d = nc.dram_tensor("gamma_rep", [128, 1], F32, kind="ExternalInput")
    out_d = nc.dram_tensor("out", [C, N], F32, kind="ExternalOutput")

    NCH = N // ICH  # input chunks per partition-half

    with tile.TileContext(nc) as tc:
        with (
            tc.tile_pool(name="singles", bufs=1) as singles,
            tc.tile_pool(name="inp", bufs=1) as inp,
            tc.tile_pool(name="hxzp", bufs=NT) as hxzp,
            tc.tile_pool(name="epool", bufs=E_BUFS) as epool,
            tc.tile_pool(name="zpool", bufs=4) as zpool,
            tc.tile_pool(name="resp", bufs=6) as resp,
            tc.tile_pool(name="outp", bufs=4) as outp,
            tc.tile_pool(name="ps_s", bufs=1, space="PSUM") as ps_s,
            tc.tile_pool(name="ps_sa", bufs=2, space="PSUM") as ps_sa,
        ):
            # ---- small constants ----
            # the f/g weights + biases + k0 ride the ACT queue (idle until the
            # first exp); wh/bh/gamma interleave on the SP queue after the
            # first own chunk; inputs: own on SP, oth on the Pool queue
            wfg_sb = [singles.tile([128, 256], F16, name=f"wfg{k}") for k in range(2)]
            wh_sb = [singles.tile([128, C], F16, name=f"wh{k}") for k in range(2)]
            bfk_sb = singles.tile([128, 3], F32)
            bh_sb = singles.tile([128, C], F32)
            gm_sb = singles.tile([128, 1], F32)
            nc.scalar.dma_start(out=bfk_sb, in_=bfk_d[:, :])
            for k in range(2):
                nc.scalar.dma_start(out=wfg_sb[k], in_=wfg_d[128 * k:128 * (k + 1), :])
            wf_sb = [w[:, 0:128] for w in wfg_sb]
            wg_sb = [w[:, 128:256] for w in wfg_sb]
            bf_sb = bfk_sb[:, 0:1]
            bg_sb = bfk_sb[:, 1:2]
            k0_sb = bfk_sb[:, 2:3]

            own_sb = [[inp.tile([128, ICH], F16, name=f"own{k}_{c}")
                       for c in range(NCH)] for k in range(2)]
            oth_sb = [[inp.tile([128, ICH], F16, name=f"oth{k}_{c}")
                       for c in range(NCH)] for k in range(2)]
            for k in range(2):
                nc.sync.dma_start(
                    out=own_sb[k][0],
                    in_=own_d[128 * k:128 * (k + 1), 0:ICH])
                nc.gpsimd.dma_start(
                    out=oth_sb[k][0],
                    in_=oth_d[128 * k:128 * (k + 1), 0:ICH])
            for k in range(2):
                nc.sync.dma_start(out=wh_sb[k], in_=wh_d[128 * k:128 * (k + 1), :])
            nc.sync.dma_start(out=bh_sb, in_=bh_d[:, :])
            nc.sync.dma_start(out=gm_sb, in_=gm_d[:, :])
            for c in range(1, NCH):
                for k in range(2):
                    nc.sync.dma_start(
                        out=own_sb[k][c],
                        in_=own_d[128 * k:128 * (k + 1), ICH * c:ICH * (c + 1)])
                    nc.gpsimd.dma_start(
                        out=oth_sb[k][c],
                        in_=oth_d[128 * k:128 * (k + 1), ICH * c:ICH * (c + 1)])

            # f/g as per-m-block tiles (dependency granularity lets group 0's
            # stats overlap the conv tail); 4 partition-group replicas each.
            f_q = [singles.tile([128, MB], F16, name=f"f{nb}") for nb in range(NMB)]
            g_q = [singles.tile([128, MB], F16, name=f"g{nb}") for nb in range(NMB)]
            sa_sb = [singles.tile([128, N], F32, name=f"sa{k}") for k in range(2)]
            hxz = [hxzp.tile([128, C], BF16, name=f"hxz{i}", tag="hxz")
                   for i in range(NT)]
            xr_sb = {}

            def conv_fg(dst, w_sb, src, b_sb, nb, ps=None):
                # weights pre-tiled 4x along output columns: the conv output
                # lands replicated in all four partition quadrants
                c, o = (MB * nb) // ICH, (MB * nb) % ICH
                if ps is None:
                    ps = ps_sa.tile([128, MB], F32, tag="sa")
                first = True
                bands = (1, 0) if BANDS else (None,)
                for k in range(2):
                    for b in bands:
                        sl = slice(None) if b is None else slice(64 * b, 64 * (b + 1))
                        nc.tensor.matmul(
                            out=ps,
                            lhsT=w_sb[k][sl, :],
                            rhs=src[k][c][sl, o:o + MB],
                            start=first,
                            stop=(k == 1 and b in (0, None)),
                            tile_position=(0 if b is None else 64 * b, 0),
                        )
                        first = False
                nc.vector.tensor_scalar(
                    out=dst[nb],
                    in0=ps,
                    scalar1=b_sb[0:128, 0:1],
                    scalar2=None,
                    op0=mybir.AluOpType.add,
                )

            def conv_h(i):
                c, o = (128 * i) // ICH, (128 * i) % ICH
                ph = ps_sa.tile([128, MB], F32, tag="sa")
                first = True
                bands = (1, 0) if BANDS else (None,)
                for k in range(2):
                    for b in bands:
                        sl = slice(None) if b is None else slice(64 * b, 64 * (b + 1))
                        nc.tensor.matmul(
                            out=ph[:, 0:C],
                            lhsT=own_sb[k][c][sl, o:o + 128],
                            rhs=wh_sb[k][sl, :],
                            start=first,
                            stop=(k == 1 and b in (0, None)),
                            tile_position=(0 if b is None else 64 * b, 0),
                        )
                        first = False
                # bias folded into the PSUM->SBUF cast (bh replicated on host)
                nc.vector.tensor_tensor(
                    out=hxz[i],
                    in0=ph[:, 0:C],
                    in1=bh_sb,
                    op=mybir.AluOpType.add,
                )

            # E half-tiles for the in-flight groups: e_half[g % 3][a][h]
            e_half = [[[None] * 2 for _ in range(NGROUP)] for _ in range(3)]

            # S PSUM: one [128, 3*SUB] tile used as a 3-slot ring of
            # [128, SUB] sub-chunks (6 banks).  Each chunk occupies two ring
            # slots; the single exp reads both through a 2-level access
            # pattern, so the full [128, 2048] chunk costs ONE activation
            # (one fixed overhead + one accum read) while the ring still
            # lets the next S pack start before the previous exp finishes.
            s_ring = ps_s.tile([128, 3 * SUB], F32, name="s_ring")
            s_r3 = s_ring.rearrange("p (c m) -> p c m", c=3)
            ring_pair = {0: s_r3[:, 0:2, :], 2: s_r3[:, 2::-2, :],
                         1: s_r3[:, 1:3, :]}
            chunk_ctr = [0]

            def stats_chunk(g, a, h, zp):
                """S chunk (n-tile 4g+a, m half h) -> exp -> E + Z part."""
                i = NGROUP * g + a
                nb, o = i // NGROUP, 128 * (i % NGROUP)
                et = epool.tile([128, HALF], BF16, name=f"e{g}_{a}_{h}", tag="e")
                e_half[g % 3][a][h] = et
                k = chunk_ctr[0]
                chunk_ctr[0] += 1
                r0 = (2 * k) % 3
                for s in range(2):
                    r = (2 * k + s) % 3
                    for jj in range(2):
                        j = 2 * s + jj
                        nc.tensor.matmul(
                            out=s_r3[:, r, MB * jj:MB * (jj + 1)],
                            lhsT=f_q[nb][32 * j:32 * (j + 1), o:o + 128],
                            rhs=g_q[4 * h + j][32 * j:32 * (j + 1), :],
                            start=True,
                            stop=True,
                            tile_position=(32 * j, 0),
                        )
                nc.scalar.activation(
                    out=et,
                    in_=ring_pair[r0],
                    func=AF.Exp,
                    bias=k0_sb[:, 0:1],
                    accum_out=zp[:, 2 * a + h:2 * a + h + 1],
                )

            def zprep(g, zp):
                """Z = sum of the two half-sums; hxz *= 1/Z (in place)."""
                zt = zpool.tile([128, NGROUP], F32, tag="zt")
                rz = zpool.tile([128, NGROUP], F32, tag="rz")
                nc.vector.tensor_add(out=zt, in0=zp[:, 0:8:2], in1=zp[:, 1:8:2])
                nc.vector.reciprocal(out=rz, in_=zt)
                # gamma folded here: hxz = h * (gamma / Z), so sa_sb directly
                # accumulates gamma*sa and the epilogue is a single add
                nc.vector.tensor_scalar(
                    out=rz, in0=rz, scalar1=gm_sb[:, 0:1], scalar2=None,
                    op0=mybir.AluOpType.mult,
                )
                for a in range(NGROUP):
                    nc.vector.tensor_scalar(
                        out=hxz[NGROUP * g + a],
                        in0=hxz[NGROUP * g + a],
                        scalar1=rz[:, a:a + 1],
                        scalar2=None,
                        op0=mybir.AluOpType.mult,
                    )

            def sa_mb(g, mb):
                """Accumulate group g's contribution to sa[:, mb block]."""
                h = mb // (NMB // 2)
                m0 = MB * mb - HALF * h
                pas = []
                for ch in range(2):
                    pa = ps_sa.tile([128, MB], F32, tag="sa")
                    first = True
                    bands = (1, 0) if BANDS else (None,)
                    for a in range(NGROUP):
                        et = e_half[g % 3][a][h]
                        hx = hxz[NGROUP * g + a]
                        for b in bands:
                            sl = slice(None) if b is None else slice(64 * b, 64 * (b + 1))
                            nc.tensor.matmul(
                                out=pa,
                                lhsT=hx[sl, 128 * ch:128 * (ch + 1)],
                                rhs=et[sl, m0:m0 + MB],
                                start=first,
                                stop=(a == NGROUP - 1 and b in (0, None)),
                                tile_position=(0 if b is None else 64 * b, 0),
                            )
                            first = False
                    dst = sa_sb[ch][:, MB * mb:MB * (mb + 1)]
                    if g == 0:
                        nc.vector.tensor_copy(out=dst, in_=pa)
                    elif g < NG - 1:
                        nc.vector.tensor_add(out=dst, in0=dst, in1=pa)
                    else:
                        pas.append(pa)
                return pas

            def prefetch_xr(mb):
                for ch in range(2):
                    xr = resp.tile([128, MB], F32, tag="xr")
                    xr_sb[(mb, ch)] = xr
                    nc.gpsimd.dma_start(
                        out=xr,
                        in_=res_d[128 * ch:128 * (ch + 1), MB * mb:MB * (mb + 1)])

            def epilogue_mb(mb, pa_last):
                # final group's PSUM merged into sa_sb, then + residual
                for ch in range(2):
                    ot = outp.tile([128, MB], F32, tag="ot")
                    nc.vector.tensor_add(
                        out=ot,
                        in0=sa_sb[ch][:, MB * mb:MB * (mb + 1)],
                        in1=pa_last[ch],
                    )
                    nc.vector.tensor_add(out=ot, in0=ot, in1=xr_sb[(mb, ch)])
                    eng = nc.sync if ch == 0 else nc.gpsimd
                    eng.dma_start(
                        out=out_d[128 * ch:128 * (ch + 1), MB * mb:MB * (mb + 1)],
                        in_=ot,
                    )

            # ---- slot schedule ----
            # Conv work not needed before the first stats chunk becomes the
            # trailing filler for the first SA_SHIFT slots; afterwards the
            # trailing sa m-blocks (lagging SA_SHIFT slots) fill that role.
            filler = [("g", nb) for nb in range(4, NMB)] \
                   + [("f", nb) for nb in range(1, NMB)] \
                   + [("h", i) for i in range(NT)]
            # upfront: everything the first stats chunk (n-tile 0, half 0)
            # needs: f block 0 and g blocks 0..3.  These use the (still idle)
            # S-ring PSUM regions so they pipeline without pool ping-pong.
            conv_fg(f_q, wf_sb, oth_sb, bf_sb, 0, ps=s_r3[:, 0, 0:MB])
            for nb in range(4):
                r, half = (1 + nb) // 2, (1 + nb) % 2
                conv_fg(g_q, wg_sb, own_sb, bg_sb, nb,
                        ps=s_r3[:, r, MB * half:MB * (half + 1)])

            fill_per_slot = (len(filler) + SA_SHIFT - 1) // SA_SHIFT
            zps = {}

            def emit_slot_filler(pos):
                sidx = pos - SA_SHIFT
                if sidx >= 0:
                    sg, smb = sidx // NMB, sidx % NMB
                    if smb == 0:
                        zprep(sg, zps.pop(sg))
                    x0 = (NG - 1) * NMB - 3
                    if x0 <= sidx < x0 + NMB:
                        prefetch_xr(sidx - x0)
                    pas = sa_mb(sg, smb)
                    if sg == NG - 1:
                        epilogue_mb(smb, pas)
                else:
                    for _ in range(fill_per_slot):
                        if filler:
                            kind, arg = filler.pop(0)
                            if kind == "g":
                                conv_fg(g_q, wg_sb, own_sb, bg_sb, arg)
                            elif kind == "f":
                                conv_fg(f_q, wf_sb, oth_sb, bf_sb, arg)
                            elif kind == "h":
                                conv_h(arg)
                            else:
                                prefetch_xr(arg)

            chunks = [(a, h) for h in range(2) for a in range(NGROUP)]
            for g in range(NG):
                zps[g] = zpool.tile([128, 2 * NGROUP], F32, tag="zp", name=f"zp{g}")
                for k, (a, h) in enumerate(chunks):
                    stats_chunk(g, a, h, zps[g])
                    emit_slot_filler(g * 8 + k)
            for pos in range(NG * 8, NG * 8 + SA_SHIFT):
                emit_slot_filler(pos)

    # run_bass_via_pjrt binds the exec primitive directly and never
    # finalizes; Bacc's register allocation + matmul-wait splitting live in
    # finalize()/compile(), so run it here.
    if not nc.is_finalized():
        nc.finalize()
    return nc


_NC_CACHE = None


def _get_nc():
    global _NC_CACHE
    if _NC_CACHE is None:
        _NC_CACHE = build_bass()
    return _NC_CACHE


def make_in_maps(**inputs):
    """Build the 8 per-core input maps (core 2b = x-branch, 2b+1 = y-branch)."""
    f = lambda a: np.ascontiguousarray(np.asarray(a), dtype=np.float32)
    h16 = lambda a: np.ascontiguousarray(np.asarray(a), dtype=np.float16)
    x = f(inputs["x"]).reshape(B, C, N)
    y = f(inputs["y"]).reshape(B, C, N)
    x16, y16 = x.astype(np.float16), y.astype(np.float16)
    Wfx, bfx = h16(inputs["Wfx"]), f(inputs["bfx"])
    Wgx, bgx = h16(inputs["Wgx"]), f(inputs["bgx"])
    Whx, bhx = h16(inputs["Whx"]), f(inputs["bhx"])
    Wfy, bfy = h16(inputs["Wfy"]), f(inputs["bfy"])
    Wgy, bgy = h16(inputs["Wgy"]), f(inputs["bgy"])
    Why, bhy = h16(inputs["Why"]), f(inputs["bhy"])
    gamma = f(inputs["gamma"])

    rep4 = lambda b: np.ascontiguousarray(np.tile(b, 4).reshape(128, 1))
    gam = np.ascontiguousarray(np.broadcast_to(gamma.reshape(1, 1), (128, 1)))

    # conv weights transposed AND 4x-tiled along output columns so the conv
    # lands replicated in the four partition quadrants
    c16 = lambda a: np.ascontiguousarray(a, dtype=np.float16)
    t4 = lambda Wt: c16(np.tile(Wt.T, (1, 4)))  # [C8, C] -> [C, 128]
    repb = lambda b: np.ascontiguousarray(
        np.broadcast_to(b.reshape(1, C), (128, C)), dtype=np.float32)
    k0_col = np.full((128, 1), -K0, np.float32)
    pack3 = lambda bf, bg: np.ascontiguousarray(
        np.concatenate([rep4(bf), rep4(bg), k0_col], axis=1), dtype=np.float32)
    packw = lambda Wf, Wg: np.ascontiguousarray(
        np.concatenate([t4(Wf), t4(Wg)], axis=1), dtype=np.float16)
    branch = {
        "x": dict(
            wfg_t=packw(Wfy, Wgx), wh_t=c16(Whx.T),
            bfgk=pack3(bfy, bgx), bh_rep=repb(bhx),
        ),
        "y": dict(
            wfg_t=packw(Wfx, Wgy), wh_t=c16(Why.T),
            bfgk=pack3(bfx, bgy), bh_rep=repb(bhy),
        ),
    }

    in_maps = []
    for b in range(B):
        in_maps.append(dict(own16=x16[b], oth16=y16[b], own32=x[b],
                            gamma_rep=gam, **branch["x"]))
        in_maps.append(dict(own16=y16[b], oth16=x16[b], own32=y[b],
                            gamma_rep=gam, **branch["y"]))
    return in_maps


def kernel(**inputs):
    from concourse.bass_utils import run_bass_kernel_spmd

    nc = _get_nc()
    in_maps = make_in_maps(**inputs)
    res = run_bass_kernel_spmd(nc, in_maps, list(range(8))).results
    out_x = np.stack([res[2 * b]["out"] for b in range(B)]).reshape(B, C, H, W)
    out_y = np.stack([res[2 * b + 1]["out"] for b in range(B)]).reshape(B, C, H, W)
    return (out_x, out_y)
